# revision 1
# baseline (speedup 1.0000x reference)
"""Trainium2 Bass kernel for nn_Direction_Attention_layer (sparse_attention).

Math (S == D == 512):
    uit  = tanh(x @ W + b)                      [B, S, D]
    a    = exp(uit @ U)                         [B, S, D]
    fw_a[b,d] = EPS + sum_{s > d} a[b,s,d]      (mask couples seq idx with feat idx)
    bw_a[b,d] = EPS + sum_{s < d} a[b,s,d]
    xs[b,d]   = sum_s x[b,s,d]
    out  = concat(fw_a * xs, bw_a * xs)         [B, 2D]

Sharding: data-parallel over batch B=64 across 8 cores (8 batches/core);
W/U/b replicated. No collectives.

Precision strategy (validated numerically): both matmuls run in bf16 (PE
streams 2 cols/cycle + fast weight load), while xs and all post-exp
accumulation stay fp32. The tanh/exp/sum structure averages out the bf16
quantization; end-to-end error is ~1.9e-4 fro-relative vs the fp32 reference,
vs ~1.1e-4 for an all-fp32r variant that runs ~3x slower on the PE.

Per-core layout is fully transposed: the host pre-transposes x to [B, D, S]
(staging, like sharding), so tiles load with 2KB-contiguous descriptors, both
matmuls keep the contraction dim on partitions, and the masked sums decompose
into:
  - full 128-col blocks left/right of the diagonal block -> fused into the
    exp activation via accum_out (free-axis sum, no extra passes)
  - the diagonal 128x128 block -> per-f DVE masked multiply + reduce
Both directions are accumulated as sums of positives (no cancellation).
"""

import sys

sys.path.insert(0, "/opt/trn_rl_repo")

import numpy as np

B, S, D = 64, 512, 512
N_CORES = 8
BPC = B // N_CORES  # batches per core
NT = D // 128  # 4 partition tiles
EPS = 1e-7

_NC_CACHE = {}


def _build_nc(repeat: int = 1):
    import concourse.bass as bass
    import concourse.tile as tile
    from concourse import bacc, mybir

    FP32 = mybir.dt.float32
    BF16 = mybir.dt.bfloat16
    AX = mybir.AxisListType
    OP = mybir.AluOpType
    AF = mybir.ActivationFunctionType

    nc = bacc.Bacc("TRN2", target_bir_lowering=False, debug=False, num_devices=N_CORES)

    # host-pre-transposed x shard [BPC, D, S], split as bf16 hi + bf16 residual:
    # hi feeds matmul 1; hi+lo reconstruct x to ~2^-17 for the xs reduction.
    # Half the DMA traffic of shipping fp32 x + a bf16 matmul copy.
    xp_ext = nc.declare_dram_parameter("xp", [BPC, 2, D, S], BF16, isOutput=False)
    w_ext = nc.declare_dram_parameter("w", [D, D], BF16, isOutput=False)
    u_ext = nc.declare_dram_parameter("u", [D, D], BF16, isOutput=False)
    b_ext = nc.declare_dram_parameter("bvec", [D], FP32, isOutput=False)
    fwm_ext = nc.declare_dram_parameter("fwmask", [128, D], FP32, isOutput=False)
    bwm_ext = nc.declare_dram_parameter("bwmask", [128, D], FP32, isOutput=False)
    o_ext = nc.declare_dram_parameter("o", [BPC, 2 * D], FP32, isOutput=True)

    with tile.TileContext(nc) as tc:
        with (
            tc.tile_pool(name="consts", bufs=1) as cpool,
            tc.tile_pool(name="xf", bufs=2) as xf_pool,
            tc.tile_pool(name="xb", bufs=3) as xb_pool,
            tc.tile_pool(name="uitt", bufs=2) as uit_pool,
            tc.tile_pool(name="diag", bufs=2) as diag_pool,
            tc.tile_pool(name="scr", bufs=2) as scr_pool,
            tc.tile_pool(name="masked", bufs=2) as msk_pool,
            tc.tile_pool(name="sums", bufs=2) as sum_pool,
            tc.tile_pool(name="outsb", bufs=2) as out_pool,
            tc.tile_pool(name="ps1", bufs=4, space="PSUM") as ps1_pool,
            tc.tile_pool(name="ps2", bufs=4, space="PSUM") as ps2_pool,
        ):
            # ---- constants ----
            w_t = cpool.tile([128, NT, D], BF16)  # w_t[p,k,e] = W[128k+p, e]
            u_t = cpool.tile([128, NT, D], BF16)
            bias = cpool.tile([128, NT], FP32)  # bias[p,e] = b[128e+p]
            fwm = cpool.tile([128, D], FP32)
            bwm = cpool.tile([128, D], FP32)

            def load_w_head():
                # bias is 2KB and gates the first tanh — land it first
                nc.sync.dma_start(
                    out=bias[:], in_=b_ext.rearrange("(e p) -> p e", p=128)
                )
                # just w[k0, e0-cols]: all the first matmul needs from W
                nc.sync.dma_start(out=w_t[:, 0, 0:128], in_=w_ext[0:128, 0:128])

            def load_w_rest():
                nc.sync.dma_start(out=w_t[:, 0, 128:D], in_=w_ext[0:128, 128:D])
                for k in range(1, NT):
                    nc.sync.dma_start(
                        out=w_t[:, k, :], in_=w_ext[128 * k : 128 * (k + 1), :]
                    )

            def load_consts_late():
                # U per-f slices: mm2(b0, f) can start as soon as slice f lands
                for f in range(NT):
                    nc.sync.dma_start(
                        out=u_t[:, :, 128 * f : 128 * (f + 1)],
                        in_=u_ext[:, 128 * f : 128 * (f + 1)].rearrange(
                            "(k p) f -> p k f", p=128
                        ),
                    )
                nc.sync.dma_start(out=fwm[:], in_=fwm_ext[:])
                nc.sync.dma_start(out=bwm[:], in_=bwm_ext[:])

            def load(b, split=False):
                """xh/xl[p,k,s] = hi/lo of x[b, s, 128k+p]; host
                pre-transposed, so contiguous-descriptor DMAs (hi split per-k
                for batch 0 so the first matmul starts earliest)."""
                xp = xb_pool.tile([128, 2, NT, S], BF16, tag="xp")
                if split:
                    # hi plane first, per-k, so the first matmul starts earliest
                    for k in range(NT):
                        nc.sync.dma_start(
                            out=xp[:, 0, k, :],
                            in_=xp_ext[b, 0, 128 * k : 128 * (k + 1), :],
                        )
                    nc.sync.dma_start(
                        out=xp[:, 1],
                        in_=xp_ext[b, 1].rearrange("(k p) s -> p k s", p=128),
                    )
                else:
                    nc.sync.dma_start(
                        out=xp[:],
                        in_=xp_ext[b].rearrange("h (k p) s -> p h k s", p=128),
                    )
                return xp[:, 1], xp[:, 0]

            def mm1(xb):
                """uitT[p,e,s] = tanh((x @ W)^T + b), written as bf16."""
                uitt = uit_pool.tile([128, NT, S], BF16, tag="uitt")
                for e in range(NT):
                    ps1 = ps1_pool.tile([128, S], FP32, tag="ps1")
                    for k in range(NT):
                        nc.tensor.matmul(
                            ps1[:],
                            lhsT=w_t[:, k, 128 * e : 128 * (e + 1)],
                            rhs=xb[:, k, :],
                            start=(k == 0),
                            stop=(k == NT - 1),
                        )
                    nc.scalar.activation(
                        uitt[:, e, :], ps1[:], AF.Tanh, bias=bias[:, e : e + 1]
                    )
                return uitt

            def mm2_post(b, xh, xl, uitt):
                """a^T = exp((uit @ U)^T); masked sums; write output row b.

                Accumulators live in [128, 8] tiles, cols 0-3 = fw per f_tile,
                cols 4-7 = bw, matching the output layout directly."""
                # xs2[p,k] = xs2[p,4+k] = sum_s x[b, s, 128k+p]
                # fp32 accumulation over bf16 hi + lo halves (~2^-17 accurate)
                xs2 = sum_pool.tile([128, 2 * NT], FP32, tag="xs2")
                xsl = sum_pool.tile([128, NT], FP32, tag="xsl")
                nc.vector.tensor_reduce(
                    out=xs2[:, 0:NT], in_=xh[:], axis=AX.X, op=OP.add
                )
                nc.vector.tensor_reduce(out=xsl[:], in_=xl[:], axis=AX.X, op=OP.add)
                nc.vector.tensor_tensor(
                    out=xs2[:, 0:NT], in0=xs2[:, 0:NT], in1=xsl[:], op=OP.add
                )
                nc.vector.tensor_copy(xs2[:, NT : 2 * NT], xs2[:, 0:NT])

                pb = sum_pool.tile([128, 2 * NT], FP32, tag="pb")
                nc.vector.memset(pb[:], 0.0)

                diag = diag_pool.tile([128, D], FP32, tag="diag")
                scr = scr_pool.tile([128, S], FP32, tag="scr")
                mfw = msk_pool.tile([128, D], FP32, tag="mfw")
                mbw = msk_pool.tile([128, D], FP32, tag="mbw")
                dd = sum_pool.tile([128, 2 * NT], FP32, tag="dd")
                for f in range(NT):
                    ps2 = ps2_pool.tile([128, S], FP32, tag="ps2")
                    for e in range(NT):
                        nc.tensor.matmul(
                            ps2[:],
                            lhsT=u_t[:, e, 128 * f : 128 * (f + 1)],
                            rhs=uitt[:, e, :],
                            start=(e == 0),
                            stop=(e == NT - 1),
                        )
                    lo, hi = 128 * f, 128 * (f + 1)
                    # diagonal 128x128 block, masked on DVE after the f-loop
                    nc.scalar.activation(diag[:, lo:hi], ps2[:, lo:hi], AF.Exp)
                    # strictly-below-diag cols: all s < d for this tile -> bw
                    if f > 0:
                        nc.scalar.activation(
                            scr[:, 0:lo],
                            ps2[:, 0:lo],
                            AF.Exp,
                            accum_out=pb[:, NT + f : NT + f + 1],
                        )
                    # strictly-above-diag cols: all s > d -> fw
                    if f < NT - 1:
                        nc.scalar.activation(
                            scr[:, hi:S],
                            ps2[:, hi:S],
                            AF.Exp,
                            accum_out=pb[:, f : f + 1],
                        )

                # masked diagonal sums, one big op per direction:
                # dd[p,f] = sum_j diag[p, 128f+j]*(j>p); dd[p,4+f] with (j<p)
                nc.vector.tensor_tensor(out=mfw[:], in0=diag[:], in1=fwm[:], op=OP.mult)
                nc.vector.tensor_reduce(
                    out=dd[:, 0:NT],
                    in_=mfw.rearrange("p (f j) -> p f j", j=128),
                    axis=AX.X,
                    op=OP.add,
                )
                nc.vector.tensor_tensor(out=mbw[:], in0=diag[:], in1=bwm[:], op=OP.mult)
                nc.vector.tensor_reduce(
                    out=dd[:, NT : 2 * NT],
                    in_=mbw.rearrange("p (f j) -> p f j", j=128),
                    axis=AX.X,
                    op=OP.add,
                )

                # out[b, half*512 + 128f + p] = (partial + diag + EPS) * xs
                osb = out_pool.tile([128, 2 * NT], FP32, tag="osb")
                nc.vector.tensor_tensor(out=osb[:], in0=pb[:], in1=dd[:], op=OP.add)
                nc.vector.tensor_scalar_add(osb[:], osb[:], EPS)
                nc.vector.tensor_tensor(out=osb[:], in0=osb[:], in1=xs2[:], op=OP.mult)
                nc.sync.dma_start(
                    out=o_ext[b].rearrange("(c p) -> p c", p=128), in_=osb[:]
                )

            # software-pipelined schedule: mm1(b) ahead of mm2(b-1) so the PE
            # never waits on the tanh chain
            def body(first_iter):
                state = []  # (b, xh, xl, uitt)
                for b in range(BPC + 1):
                    if b < BPC:
                        if b == 0 and first_iter:
                            load_w_head()
                        xl, xh = load(b, split=(b == 0 and first_iter))
                        if b == 0 and first_iter:
                            load_w_rest()
                            load_consts_late()
                        uitt = mm1(xh)
                        state.append((b, xh, xl, uitt))
                    if b >= 1:
                        pb_, pxh, pxl, puitt = state[b - 1]
                        mm2_post(pb_, pxh, pxl, puitt)

            if repeat == 1:
                body(True)
            else:
                # benchmarking mode: repeat the whole computation on-device in
                # a hardware loop so per-iteration time is measurable above
                # host/axon dispatch noise
                load_w_head()
                load_w_rest()
                load_consts_late()
                with tc.For_i(0, repeat, 1):
                    body(False)

    nc.finalize()
    return nc


def _make_mask_inputs():
    j = np.arange(128, dtype=np.int64)
    blk_fw = (j[None, :] > j[:, None]).astype(np.float32)  # j > p
    blk_bw = (j[None, :] < j[:, None]).astype(np.float32)  # j < p
    return np.tile(blk_fw, (1, NT)), np.tile(blk_bw, (1, NT))


def _make_in_maps(x, W, U, b):
    import ml_dtypes

    # host staging: shard, pre-transpose to [B, D, S], split into bf16 hi+lo
    xt = np.ascontiguousarray(
        np.asarray(x, dtype=np.float32).transpose(0, 2, 1)
    )
    xh = xt.astype(ml_dtypes.bfloat16)
    xl = (xt - xh.astype(np.float32)).astype(ml_dtypes.bfloat16)
    xp = np.ascontiguousarray(np.stack([xh, xl], axis=1))
    wb = np.asarray(W, dtype=np.float32).astype(ml_dtypes.bfloat16)
    ub = np.asarray(U, dtype=np.float32).astype(ml_dtypes.bfloat16)
    bf = np.ascontiguousarray(b, dtype=np.float32)
    fwmask, bwmask = _make_mask_inputs()
    return [
        {
            "xp": xp[c * BPC : (c + 1) * BPC],
            "w": wb,
            "u": ub,
            "bvec": bf,
            "fwmask": fwmask,
            "bwmask": bwmask,
        }
        for c in range(N_CORES)
    ]


def kernel(x, W, U, b):
    from concourse.bass_utils import run_bass_kernel_spmd

    x = np.asarray(x)
    assert x.shape == (B, S, D)
    key = "nc"
    if key not in _NC_CACHE:
        _NC_CACHE[key] = _build_nc()
    nc = _NC_CACHE[key]

    in_maps = _make_in_maps(x, np.asarray(W), np.asarray(U), np.asarray(b))
    res = run_bass_kernel_spmd(nc, in_maps, list(range(N_CORES)))
    out = np.concatenate([res.results[c]["o"] for c in range(N_CORES)], axis=0)
    return out.astype(np.float32)



# revision 2
# speedup vs baseline: 1.0983x; 1.0983x over previous
"""Trainium2 Bass kernel for nn_Direction_Attention_layer (sparse_attention), v3.

Math (S == D == 512):
    uit  = tanh(x @ W + b);  a = exp(uit @ U)
    fw_a[d] = EPS + sum_{s>d} a[s,d];  bw_a[d] = EPS + sum_{s<d} a[s,d]
    out = concat(fw_a * xs, bw_a * xs),  xs[d] = sum_s x[s,d]

Sharding: data-parallel over batch B=64 across 8 cores; W/U replicated.

v3 engine plan (per batch, per core; measured HW rates):
- PE: both GEMMs fp8 e4m3 DoubleRow (K=256/MM): 16 MMs x ~240ns = 3.84us
- ACT: ONE tanh [128,2048] PSUM->fp8 and ONE exp [128,2048] PSUM->bf16
  (~1.85us each; per-instr init is the TRN2 SBUF-src errata cost)
- DVE: avoids the ~(dur-266ns) pipe-drain tax by using fold-add trees of
  small bf16 ops (2x rate, each op < 266ns after the first level):
    * bs block sums:   tail of the Pool fold1 -> [128,16] fp32
    * diag triangular: 2 masked bf16 mults on a stride-640 diagonal view
      + fold tree -> [128,8] fp32
    * xs: bf16 fold tree on xh + small fp32 reduce tail
- Pool (gpsimd): fold1 of the bs tree + the [128,16]-sized assembly ops.
W/U pre-scaled by 2^11 into e4m3 normal range; descale via activation scale.
"""

import sys

sys.path.insert(0, "/opt/trn_rl_repo")

import numpy as np

B, S, D = 64, 512, 512
N_CORES = 8
BPC = B // N_CORES
NT = D // 128  # 4
EPS = 1e-7
WSCALE = 2048.0

_NC_CACHE = {}


def _build_nc(repeat: int = 1, with_bias: bool = False, unroll: int = 1):
    import concourse.bass as bass
    import concourse.tile as tile
    from concourse import bacc, mybir

    FP32 = mybir.dt.float32
    BF16 = mybir.dt.bfloat16
    FP8 = mybir.dt.float8e4
    AX = mybir.AxisListType
    OP = mybir.AluOpType
    AF = mybir.ActivationFunctionType
    DR = mybir.MatmulPerfMode.DoubleRow

    nc = bacc.Bacc("TRN2", target_bir_lowering=False, debug=False, num_devices=N_CORES)

    x8_ext = nc.declare_dram_parameter("x8", [BPC, 128, NT, S], FP8, isOutput=False)
    xh_ext = nc.declare_dram_parameter("xh", [BPC, 128, NT, S], BF16, isOutput=False)
    w8_ext = nc.declare_dram_parameter("w8", [128, 2, 2, D], FP8, isOutput=False)
    u8_ext = nc.declare_dram_parameter("u8", [128, 2, 2, D], FP8, isOutput=False)
    dm_ext = nc.declare_dram_parameter("dmask", [128, 2, NT, 128], BF16, isOutput=False)
    bm_ext = nc.declare_dram_parameter("bmask", [128, 2, NT, NT], FP32, isOutput=False)
    if with_bias:
        b_ext = nc.declare_dram_parameter("bvec", [D], FP32, isOutput=False)
    o_ext = nc.declare_dram_parameter("o", [BPC, 2 * D], FP32, isOutput=True)

    with tile.TileContext(nc) as tc:
        with (
            tc.tile_pool(name="consts", bufs=1) as cpool,
            tc.tile_pool(name="x8p", bufs=4) as x8_pool,
            tc.tile_pool(name="xhp", bufs=4) as xh_pool,
            tc.tile_pool(name="uitt", bufs=3) as uit_pool,
            tc.tile_pool(name="at", bufs=3) as at_pool,
            tc.tile_pool(name="mid", bufs=4) as mid_pool,
            tc.tile_pool(name="sums", bufs=4) as sum_pool,
            tc.tile_pool(name="ps1", bufs=1, space="PSUM") as ps1_pool,
            tc.tile_pool(name="ps2", bufs=1, space="PSUM") as ps2_pool,
        ):
            w8 = cpool.tile([128, 2, 2, D], FP8)
            u8 = cpool.tile([128, 2, 2, D], FP8)
            dmask = cpool.tile([128, 2, NT, 128], BF16)
            bmask = cpool.tile([128, 2, NT, NT], FP32)
            if with_bias:
                bias = cpool.tile([128, NT], FP32)

            def load_consts():
                nc.sync.dma_start(out=w8[:], in_=w8_ext[:])
                nc.sync.dma_start(out=u8[:], in_=u8_ext[:])
                nc.sync.dma_start(out=dmask[:], in_=dm_ext[:])
                nc.sync.dma_start(out=bmask[:], in_=bm_ext[:])
                if with_bias:
                    nc.sync.dma_start(
                        out=bias[:], in_=b_ext.rearrange("(e p) -> p e", p=128)
                    )

            def load(b):
                x8 = x8_pool.tile([128, NT, S], FP8, tag="x8")
                xh = xh_pool.tile([128, NT, S], BF16, tag="xh")
                nc.sync.dma_start(out=x8[:], in_=x8_ext[b])
                nc.sync.dma_start(out=xh[:], in_=xh_ext[b])
                return x8, xh

            def mm1(x8):
                uitt = uit_pool.tile([128, NT, S], FP8, tag="uitt")
                ps1 = ps1_pool.tile([128, NT, S], FP32, tag="ps1")
                for e in range(NT):
                    for j in range(2):
                        nc.tensor.matmul(
                            ps1[:, e, :],
                            lhsT=w8[:, j, :, 128 * e : 128 * (e + 1)],
                            rhs=x8[:, 2 * j : 2 * j + 2, :],
                            start=(j == 0),
                            stop=(j == 1),
                            perf_mode=DR,
                        )
                if with_bias:
                    for e in range(NT):
                        nc.scalar.activation(
                            uitt[:, e, :],
                            ps1[:, e, :],
                            AF.Tanh,
                            bias=bias[:, e : e + 1],
                            scale=float(1.0 / WSCALE),
                        )
                else:
                    nc.scalar.activation(
                        uitt.rearrange("p k s -> p (k s)"),
                        ps1.rearrange("p k s -> p (k s)"),
                        AF.Tanh,
                        scale=float(1.0 / WSCALE),
                    )
                return uitt

            def mm2_post(b, xh, uitt):
                aT = at_pool.tile([128, 5, S], BF16, tag="at")
                aflat = aT.rearrange("p k s -> p (k s)")
                ps2 = ps2_pool.tile([128, NT, S], FP32, tag="ps2")
                for f in range(NT):
                    for j in range(2):
                        nc.tensor.matmul(
                            ps2[:, f, :],
                            lhsT=u8[:, j, :, 128 * f : 128 * (f + 1)],
                            rhs=uitt[:, 2 * j : 2 * j + 2, :],
                            start=(j == 0),
                            stop=(j == 1),
                            perf_mode=DR,
                        )
                nc.scalar.activation(
                    aflat[:, 0:2048],
                    ps2.rearrange("p k s -> p (k s)"),
                    AF.Exp,
                    scale=float(1.0 / WSCALE),
                )

                # ---- xs: bf16 fold tree on DVE + small fp32 reduce tail ----
                xs2 = sum_pool.tile([128, 2 * NT], FP32, tag="xs2")
                xa = mid_pool.tile([128, NT, 256], BF16, tag="xa")
                nc.vector.tensor_tensor(
                    out=xa[:], in0=xh[:, :, 0:256], in1=xh[:, :, 256:512], op=OP.add
                )
                xb_ = mid_pool.tile([128, NT, 128], BF16, tag="xb_")
                nc.vector.tensor_tensor(
                    out=xb_[:], in0=xa[:, :, 0:128], in1=xa[:, :, 128:256], op=OP.add
                )
                xc = mid_pool.tile([128, NT, 64], BF16, tag="xc")
                nc.vector.tensor_tensor(
                    out=xc[:], in0=xb_[:, :, 0:64], in1=xb_[:, :, 64:128], op=OP.add
                )
                xd = mid_pool.tile([128, NT, 32], BF16, tag="xd")
                nc.vector.tensor_tensor(
                    out=xd[:], in0=xc[:, :, 0:32], in1=xc[:, :, 32:64], op=OP.add
                )
                nc.vector.tensor_reduce(
                    out=xs2[:, 0:NT], in_=xd[:], axis=AX.X, op=OP.add
                )
                nc.gpsimd.tensor_copy(xs2[:, NT : 2 * NT], xs2[:, 0:NT])

                # ---- bs block sums: Pool fold1, DVE bf16 tail ----
                v = aflat.rearrange("p (g j) -> p g j", j=128)[:, 0:16, :]
                bf1 = mid_pool.tile([128, 16, 64], BF16, tag="bf1")
                nc.gpsimd.tensor_tensor(
                    out=bf1[:], in0=v[:, :, 0:64], in1=v[:, :, 64:128], op=OP.add
                )
                bf2 = mid_pool.tile([128, 16, 32], BF16, tag="bf2")
                nc.vector.tensor_tensor(
                    out=bf2[:], in0=bf1[:, :, 0:32], in1=bf1[:, :, 32:64], op=OP.add
                )
                bf3 = mid_pool.tile([128, 16, 16], BF16, tag="bf3")
                nc.vector.tensor_tensor(
                    out=bf3[:], in0=bf2[:, :, 0:16], in1=bf2[:, :, 16:32], op=OP.add
                )
                bs = sum_pool.tile([128, 16], FP32, tag="bs")
                nc.vector.tensor_reduce(out=bs[:], in_=bf3[:], axis=AX.X, op=OP.add)

                # ---- diag triangular: strided view, 2 bf16 mults, fold tree ----
                dv = aflat.rearrange("p (f y) -> p f y", y=640)[:, :, 0:128]
                md = mid_pool.tile([128, 2, NT, 128], BF16, tag="md")
                for d_ in range(2):
                    nc.vector.tensor_tensor(
                        out=md[:, d_], in0=dv, in1=dmask[:, d_], op=OP.mult
                    )
                mdv = md.rearrange("p d f j -> p (d f) j")
                df1 = mid_pool.tile([128, 8, 64], BF16, tag="df1")
                nc.vector.tensor_tensor(
                    out=df1[:], in0=mdv[:, :, 0:64], in1=mdv[:, :, 64:128], op=OP.add
                )
                df2 = mid_pool.tile([128, 8, 32], BF16, tag="df2")
                nc.vector.tensor_tensor(
                    out=df2[:], in0=df1[:, :, 0:32], in1=df1[:, :, 32:64], op=OP.add
                )
                dd = sum_pool.tile([128, 2 * NT], FP32, tag="dd")
                nc.vector.tensor_reduce(out=dd[:], in_=df2[:], axis=AX.X, op=OP.add)

                # ---- assembly on Pool: mask bs, fold kb, add dd/EPS, mult xs ----
                bsm = mid_pool.tile([128, 2, NT, NT], FP32, tag="bsm")
                for d_ in range(2):
                    nc.gpsimd.tensor_tensor(
                        out=bsm[:, d_],
                        in0=bs.rearrange("p (f k) -> p f k", k=NT),
                        in1=bmask[:, d_],
                        op=OP.mult,
                    )
                of1 = mid_pool.tile([128, 2, NT, 2], FP32, tag="of1")
                nc.gpsimd.tensor_tensor(
                    out=of1[:], in0=bsm[:, :, :, 0:2], in1=bsm[:, :, :, 2:4], op=OP.add
                )
                osb = sum_pool.tile([128, 2 * NT], FP32, tag="osb")
                nc.gpsimd.tensor_tensor(
                    out=osb.rearrange("p (d f) -> p d f", d=2),
                    in0=of1[:, :, :, 0:1].rearrange("p d f o -> p d (f o)"),
                    in1=of1[:, :, :, 1:2].rearrange("p d f o -> p d (f o)"),
                    op=OP.add,
                )
                o2 = sum_pool.tile([128, 2 * NT], FP32, tag="o2")
                nc.gpsimd.tensor_tensor(out=o2[:], in0=osb[:], in1=dd[:], op=OP.add)
                nc.gpsimd.tensor_scalar_add(o2[:], o2[:], EPS)
                nc.gpsimd.tensor_tensor(out=o2[:], in0=o2[:], in1=xs2[:], op=OP.mult)
                nc.sync.dma_start(
                    out=o_ext[b].rearrange("(c p) -> p c", p=128), in_=o2[:]
                )

            def body(first_iter):
                state = []
                for b in range(BPC + 1):
                    if b < BPC:
                        if b == 0 and first_iter:
                            load_consts()
                        x8, xh = load(b)
                        uitt = mm1(x8)
                        state.append((b, xh, uitt))
                    if b >= 1:
                        pb_, pxh, puitt = state[b - 1]
                        mm2_post(pb_, pxh, puitt)

            if repeat == 1:
                body(True)
            else:
                load_consts()
                with tc.For_i(0, repeat, 1):
                    for _u in range(unroll):
                        body(False)

    nc.finalize()
    return nc


def _e4m3(a):
    import ml_dtypes

    return np.clip(np.asarray(a, np.float32), -240.0, 240.0).astype(
        ml_dtypes.float8_e4m3
    )


def _make_mask_inputs():
    import ml_dtypes

    j = np.arange(128)
    fw = (j[None, :] > j[:, None]).astype(np.float32)
    bw = fw.T
    dmask = np.stack(
        [np.repeat(fw[:, None, :], NT, axis=1), np.repeat(bw[:, None, :], NT, axis=1)]
    ).transpose(1, 0, 2, 3)
    f = np.arange(NT)
    bfw = (f[None, :] > f[:, None]).astype(np.float32)
    bbw = bfw.T
    bmask = np.broadcast_to(np.stack([bfw, bbw])[None], (128, 2, NT, NT))
    return (
        np.ascontiguousarray(dmask.astype(ml_dtypes.bfloat16)),
        np.ascontiguousarray(bmask.astype(np.float32)),
    )


def _make_in_maps(x, W, U, b):
    import ml_dtypes

    x = np.asarray(x, np.float32)
    xt = x.transpose(0, 2, 1).reshape(B, NT, 128, S).transpose(0, 2, 1, 3)
    x8 = np.ascontiguousarray(_e4m3(xt))
    xh = np.ascontiguousarray(xt.astype(ml_dtypes.bfloat16))

    def prep_w(M):
        M8 = _e4m3(np.asarray(M, np.float32) * WSCALE)
        return np.ascontiguousarray(M8.reshape(2, 2, 128, D).transpose(2, 0, 1, 3))

    w8 = prep_w(W)
    u8 = prep_w(U)
    dmask, bmask = _make_mask_inputs()
    base = {"w8": w8, "u8": u8, "dmask": dmask, "bmask": bmask}
    bvec = np.ascontiguousarray(np.asarray(b, np.float32))
    if np.any(bvec):
        base["bvec"] = bvec
    return [
        {"x8": x8[c * BPC : (c + 1) * BPC], "xh": xh[c * BPC : (c + 1) * BPC], **base}
        for c in range(N_CORES)
    ]


def kernel(x, W, U, b):
    from concourse.bass_utils import run_bass_kernel_spmd

    x = np.asarray(x)
    assert x.shape == (B, S, D)
    in_maps = _make_in_maps(x, np.asarray(W), np.asarray(U), np.asarray(b))
    with_bias = "bvec" in in_maps[0]
    key = ("nc", with_bias)
    if key not in _NC_CACHE:
        _NC_CACHE[key] = _build_nc(1, with_bias=with_bias)
    nc = _NC_CACHE[key]

    res = run_bass_kernel_spmd(nc, in_maps, list(range(N_CORES)))
    out = np.concatenate([res.results[c]["o"] for c in range(N_CORES)], axis=0)
    return out.astype(np.float32)


# revision 3
# speedup vs baseline: 1.1125x; 1.0129x over previous
"""Trainium2 Bass kernel for nn_Direction_Attention_layer (sparse_attention), v5.

Math (S == D == 512):
    uit  = tanh(x @ W + b);  a = exp(uit @ U)
    fw_a[d] = EPS + sum_{s>d} a[s,d];  bw_a[d] = EPS + sum_{s<d} a[s,d]
    out = concat(fw_a * xs, bw_a * xs),  xs[d] = sum_s x[s,d]

Sharding: data-parallel over batch B=64 across 8 cores; W/U replicated.

v5 vs v3 (both fp8-DoubleRow GEMMs + merged tanh/exp):
- ALL reduction work on DVE (measured: gpsimd tensor ops are ~4x slower
  than the cost model; independent back-to-back DVE ops run at model rate).
- One shared fold tree for block-sums + masked diagonal ([128,24,64] ->
  [128,24,32] -> [128,24] fp32).
- Stage-decoupled emission so the in-order DVE queue never waits:
  per iteration: asm(b-2) [old deps] -> folds(b-1) [dep exp(b-1)] ->
  xs folds(b-1) [dep xh(b-1), loaded last iteration].
- DMA spread across queues: x8 on sync HWDGE, xh halves on two gpsimd
  SWDGE chains (gpsimd is otherwise idle) - per-queue DMA bandwidth
  (~22GB/s) was a serialization risk at 768KB/iteration.
"""

import sys

sys.path.insert(0, "/opt/trn_rl_repo")

import numpy as np

B, S, D = 64, 512, 512
N_CORES = 8
BPC = B // N_CORES
NT = D // 128  # 4
EPS = 1e-7
WSCALE = 2048.0

_NC_CACHE = {}


def _build_nc(repeat: int = 1, with_bias: bool = False, unroll: int = 1):
    import concourse.bass as bass
    import concourse.tile as tile
    from concourse import bacc, mybir

    FP32 = mybir.dt.float32
    BF16 = mybir.dt.bfloat16
    FP8 = mybir.dt.float8e4
    AX = mybir.AxisListType
    OP = mybir.AluOpType
    AF = mybir.ActivationFunctionType
    DR = mybir.MatmulPerfMode.DoubleRow

    nc = bacc.Bacc("TRN2", target_bir_lowering=False, debug=False, num_devices=N_CORES)

    x8_ext = nc.declare_dram_parameter("x8", [BPC, 128, NT, S], FP8, isOutput=False)
    xh_ext = nc.declare_dram_parameter("xh", [BPC, 128, NT, S], BF16, isOutput=False)
    w8_ext = nc.declare_dram_parameter("w8", [128, 2, 2, D], FP8, isOutput=False)
    u8_ext = nc.declare_dram_parameter("u8", [128, 2, 2, D], FP8, isOutput=False)
    dm_ext = nc.declare_dram_parameter("dmask", [128, 2, NT, 128], BF16, isOutput=False)
    bm_ext = nc.declare_dram_parameter("bmask", [128, 2, NT, NT], FP32, isOutput=False)
    if with_bias:
        b_ext = nc.declare_dram_parameter("bvec", [D], FP32, isOutput=False)
    o_ext = nc.declare_dram_parameter("o", [BPC, 2 * D], FP32, isOutput=True)

    with tile.TileContext(nc) as tc:
        with (
            tc.tile_pool(name="consts", bufs=1) as cpool,
            tc.tile_pool(name="x8p", bufs=4) as x8_pool,
            tc.tile_pool(name="xhp", bufs=4) as xh_pool,
            tc.tile_pool(name="uitt", bufs=3) as uit_pool,
            tc.tile_pool(name="at", bufs=3) as at_pool,
            tc.tile_pool(name="mid", bufs=3) as mid_pool,
            tc.tile_pool(name="sums", bufs=4) as sum_pool,
            tc.tile_pool(name="ps1", bufs=1, space="PSUM") as ps1_pool,
            tc.tile_pool(name="ps2", bufs=1, space="PSUM") as ps2_pool,
        ):
            w8 = cpool.tile([128, 2, 2, D], FP8)
            u8 = cpool.tile([128, 2, 2, D], FP8)
            dmask = cpool.tile([128, 2, NT, 128], BF16)
            bmask = cpool.tile([128, 2, NT, NT], FP32)
            if with_bias:
                bias = cpool.tile([128, NT], FP32)

            def load_consts():
                nc.sync.dma_start(out=w8[:], in_=w8_ext[:])
                nc.sync.dma_start(out=u8[:], in_=u8_ext[:])
                nc.sync.dma_start(out=dmask[:], in_=dm_ext[:])
                nc.sync.dma_start(out=bmask[:], in_=bm_ext[:])
                if with_bias:
                    nc.sync.dma_start(
                        out=bias[:], in_=b_ext.rearrange("(e p) -> p e", p=128)
                    )

            def load(b):
                x8 = x8_pool.tile([128, NT, S], FP8, tag="x8")
                xh = xh_pool.tile([128, NT, S], BF16, tag="xh")
                nc.sync.dma_start(out=x8[:], in_=x8_ext[b])
                # xh on two gpsimd SWDGE chains (separate queues from sync)
                nc.gpsimd.dma_start(out=xh[:, 0:2, :], in_=xh_ext[b, :, 0:2, :])
                nc.gpsimd.dma_start(out=xh[:, 2:4, :], in_=xh_ext[b, :, 2:4, :])
                return x8, xh

            def mm1(x8):
                uitt = uit_pool.tile([128, NT, S], FP8, tag="uitt")
                ps1 = ps1_pool.tile([128, NT, S], FP32, tag="ps1")
                for e in range(NT):
                    for j in range(2):
                        nc.tensor.matmul(
                            ps1[:, e, :],
                            lhsT=w8[:, j, :, 128 * e : 128 * (e + 1)],
                            rhs=x8[:, 2 * j : 2 * j + 2, :],
                            start=(j == 0),
                            stop=(j == 1),
                            perf_mode=DR,
                        )
                if with_bias:
                    for e in range(NT):
                        nc.scalar.activation(
                            uitt[:, e, :],
                            ps1[:, e, :],
                            AF.Tanh,
                            bias=bias[:, e : e + 1],
                            scale=float(1.0 / WSCALE),
                        )
                else:
                    nc.scalar.activation(
                        uitt.rearrange("p k s -> p (k s)"),
                        ps1.rearrange("p k s -> p (k s)"),
                        AF.Tanh,
                        scale=float(1.0 / WSCALE),
                    )
                return uitt

            def mm2_exp(uitt):
                aT = at_pool.tile([128, 5, S], BF16, tag="at")
                aflat = aT.rearrange("p k s -> p (k s)")
                ps2 = ps2_pool.tile([128, NT, S], FP32, tag="ps2")
                for f in range(NT):
                    for j in range(2):
                        nc.tensor.matmul(
                            ps2[:, f, :],
                            lhsT=u8[:, j, :, 128 * f : 128 * (f + 1)],
                            rhs=uitt[:, 2 * j : 2 * j + 2, :],
                            start=(j == 0),
                            stop=(j == 1),
                            perf_mode=DR,
                        )
                nc.scalar.activation(
                    aflat[:, 0:2048],
                    ps2.rearrange("p k s -> p (k s)"),
                    AF.Exp,
                    scale=float(1.0 / WSCALE),
                )
                return aT

            def folds(aT):
                """Shared fold tree: bs24[:, 0:16] = block sums,
                bs24[:, 16:24] = (fw, bw) masked diagonal sums."""
                aflat = aT.rearrange("p k s -> p (k s)")
                v = aflat.rearrange("p (g j) -> p g j", j=128)[:, 0:16, :]
                cmb = mid_pool.tile([128, 24, 64], BF16, tag="cmb")
                nc.vector.tensor_tensor(
                    out=cmb[:, 0:16, :], in0=v[:, :, 0:64], in1=v[:, :, 64:128],
                    op=OP.add,
                )
                dv = aflat.rearrange("p (f y) -> p f y", y=640)[:, :, 0:128]
                md = mid_pool.tile([128, 2, NT, 128], BF16, tag="md")
                for d_ in range(2):
                    nc.vector.tensor_tensor(
                        out=md[:, d_], in0=dv, in1=dmask[:, d_], op=OP.mult
                    )
                mdv = md.rearrange("p d f j -> p (d f) j")
                nc.vector.tensor_tensor(
                    out=cmb[:, 16:24, :], in0=mdv[:, :, 0:64], in1=mdv[:, :, 64:128],
                    op=OP.add,
                )
                cm2 = mid_pool.tile([128, 24, 32], BF16, tag="cm2")
                nc.vector.tensor_tensor(
                    out=cm2[:], in0=cmb[:, :, 0:32], in1=cmb[:, :, 32:64], op=OP.add
                )
                bs24 = sum_pool.tile([128, 24], FP32, tag="bs24")
                nc.vector.tensor_reduce(out=bs24[:], in_=cm2[:], axis=AX.X, op=OP.add)
                return bs24

            def xs_folds(xh):
                xs2 = sum_pool.tile([128, 2 * NT], FP32, tag="xs2")
                xa = mid_pool.tile([128, NT, 256], BF16, tag="xa")
                nc.vector.tensor_tensor(
                    out=xa[:], in0=xh[:, :, 0:256], in1=xh[:, :, 256:512], op=OP.add
                )
                xb_ = mid_pool.tile([128, NT, 128], BF16, tag="xb_")
                nc.vector.tensor_tensor(
                    out=xb_[:], in0=xa[:, :, 0:128], in1=xa[:, :, 128:256], op=OP.add
                )
                nc.vector.tensor_reduce(
                    out=xs2[:, 0:NT], in_=xb_[:], axis=AX.X, op=OP.add
                )
                nc.vector.tensor_copy(xs2[:, NT : 2 * NT], xs2[:, 0:NT])
                return xs2

            def asm_out(b, xs2, bs24):
                bsm = mid_pool.tile([128, 2, NT, NT], FP32, tag="bsm")
                for d_ in range(2):
                    nc.vector.tensor_tensor(
                        out=bsm[:, d_],
                        in0=bs24[:, 0:16].rearrange("p (f k) -> p f k", k=NT),
                        in1=bmask[:, d_],
                        op=OP.mult,
                    )
                osb = sum_pool.tile([128, 2 * NT], FP32, tag="osb")
                nc.vector.tensor_reduce(
                    out=osb[:],
                    in_=bsm.rearrange("p d f k -> p (d f) k"),
                    axis=AX.X,
                    op=OP.add,
                )
                o2a = sum_pool.tile([128, 2 * NT], FP32, tag="o2a")
                nc.vector.tensor_tensor(
                    out=o2a[:], in0=osb[:], in1=bs24[:, 16:24], op=OP.add
                )
                o2 = sum_pool.tile([128, 2 * NT], FP32, tag="o2")
                nc.vector.scalar_tensor_tensor(
                    out=o2[:], in0=o2a[:], scalar=EPS, in1=xs2[:],
                    op0=OP.add, op1=OP.mult,
                )
                nc.sync.dma_start(
                    out=o_ext[b].rearrange("(c p) -> p c", p=128), in_=o2[:]
                )

            def body(first_iter):
                state = {}  # b -> (uitt, xh)
                mid = {}  # b -> (xs2, bs24)
                for b in range(BPC + 2):
                    if b >= 2:
                        pb = b - 2
                        asm_out(pb, *mid[pb])
                    if b < BPC:
                        if b == 0 and first_iter:
                            load_consts()
                        x8, xh = load(b)
                        uitt = mm1(x8)
                        state[b] = (uitt, xh)
                    if 1 <= b <= BPC:
                        pb = b - 1
                        uitt, xh = state[pb]
                        aT = mm2_exp(uitt)
                        bs24 = folds(aT)
                        xs2 = xs_folds(xh)
                        mid[pb] = (xs2, bs24)

            if repeat == 1:
                body(True)
            else:
                load_consts()
                with tc.For_i(0, repeat, 1):
                    for _u in range(unroll):
                        body(False)

    nc.finalize()
    return nc


def _e4m3(a):
    import ml_dtypes

    return np.clip(np.asarray(a, np.float32), -240.0, 240.0).astype(
        ml_dtypes.float8_e4m3
    )


def _make_mask_inputs():
    import ml_dtypes

    j = np.arange(128)
    fw = (j[None, :] > j[:, None]).astype(np.float32)
    bw = fw.T
    dmask = np.stack(
        [np.repeat(fw[:, None, :], NT, axis=1), np.repeat(bw[:, None, :], NT, axis=1)]
    ).transpose(1, 0, 2, 3)
    f = np.arange(NT)
    bfw = (f[None, :] > f[:, None]).astype(np.float32)
    bbw = bfw.T
    bmask = np.broadcast_to(np.stack([bfw, bbw])[None], (128, 2, NT, NT))
    return (
        np.ascontiguousarray(dmask.astype(ml_dtypes.bfloat16)),
        np.ascontiguousarray(bmask.astype(np.float32)),
    )


def _make_in_maps(x, W, U, b):
    import ml_dtypes

    x = np.asarray(x, np.float32)
    xt = x.transpose(0, 2, 1).reshape(B, NT, 128, S).transpose(0, 2, 1, 3)
    x8 = np.ascontiguousarray(_e4m3(xt))
    xh = np.ascontiguousarray(xt.astype(ml_dtypes.bfloat16))

    def prep_w(M):
        M8 = _e4m3(np.asarray(M, np.float32) * WSCALE)
        return np.ascontiguousarray(M8.reshape(2, 2, 128, D).transpose(2, 0, 1, 3))

    w8 = prep_w(W)
    u8 = prep_w(U)
    dmask, bmask = _make_mask_inputs()
    base = {"w8": w8, "u8": u8, "dmask": dmask, "bmask": bmask}
    bvec = np.ascontiguousarray(np.asarray(b, np.float32))
    if np.any(bvec):
        base["bvec"] = bvec
    return [
        {"x8": x8[c * BPC : (c + 1) * BPC], "xh": xh[c * BPC : (c + 1) * BPC], **base}
        for c in range(N_CORES)
    ]


def kernel(x, W, U, b):
    from concourse.bass_utils import run_bass_kernel_spmd

    x = np.asarray(x)
    assert x.shape == (B, S, D)
    in_maps = _make_in_maps(x, np.asarray(W), np.asarray(U), np.asarray(b))
    with_bias = "bvec" in in_maps[0]
    key = ("nc", with_bias)
    if key not in _NC_CACHE:
        _NC_CACHE[key] = _build_nc(1, with_bias=with_bias)
    nc = _NC_CACHE[key]

    res = run_bass_kernel_spmd(nc, in_maps, list(range(N_CORES)))
    out = np.concatenate([res.results[c]["o"] for c in range(N_CORES)], axis=0)
    return out.astype(np.float32)


# revision 4
# speedup vs baseline: 1.1191x; 1.0059x over previous
"""Trainium2 Bass kernel for nn_Direction_Attention_layer (sparse_attention), v6.

Math (S == D == 512):
    uit  = tanh(x @ W + b);  a = exp(uit @ U)
    fw_a[d] = EPS + sum_{s>d} a[s,d];  bw_a[d] = EPS + sum_{s<d} a[s,d]
    out = concat(fw_a * xs, bw_a * xs),  xs[d] = sum_s x[s,d]

Sharding: data-parallel over batch B=64 across 8 cores; W/U replicated.

v5 vs v3 (both fp8-DoubleRow GEMMs + merged tanh/exp):
- ALL reduction work on DVE (measured: gpsimd tensor ops are ~4x slower
  than the cost model; independent back-to-back DVE ops run at model rate).
- One shared fold tree for block-sums + masked diagonal ([128,24,64] ->
  [128,24,32] -> [128,24] fp32).
- Stage-decoupled emission so the in-order DVE queue never waits:
  per iteration: asm(b-2) [old deps] -> folds(b-1) [dep exp(b-1)] ->
  xs folds(b-1) [dep xh(b-1), loaded last iteration].
- DMA spread across queues: x8 on sync HWDGE, xh halves on two gpsimd
  SWDGE chains (gpsimd is otherwise idle) - per-queue DMA bandwidth
  (~22GB/s) was a serialization risk at 768KB/iteration.
"""

import sys

sys.path.insert(0, "/opt/trn_rl_repo")

import numpy as np

B, S, D = 64, 512, 512
N_CORES = 8
BPC = B // N_CORES
NT = D // 128  # 4
EPS = 1e-7
WSCALE = 2048.0

_NC_CACHE = {}


def _build_nc(repeat: int = 1, with_bias: bool = False, unroll: int = 1):
    import concourse.bass as bass
    import concourse.tile as tile
    from concourse import bacc, mybir

    FP32 = mybir.dt.float32
    BF16 = mybir.dt.bfloat16
    FP8 = mybir.dt.float8e4
    AX = mybir.AxisListType
    OP = mybir.AluOpType
    AF = mybir.ActivationFunctionType
    DR = mybir.MatmulPerfMode.DoubleRow

    nc = bacc.Bacc("TRN2", target_bir_lowering=False, debug=False, num_devices=N_CORES, num_swdge_queues=4)

    x8_ext = nc.declare_dram_parameter("x8", [BPC, 128, NT, S], FP8, isOutput=False)
    xh_ext = nc.declare_dram_parameter("xh", [BPC, 128, NT, S], BF16, isOutput=False)
    w8_ext = nc.declare_dram_parameter("w8", [128, 2, 2, D], FP8, isOutput=False)
    u8_ext = nc.declare_dram_parameter("u8", [128, 2, 2, D], FP8, isOutput=False)
    dm_ext = nc.declare_dram_parameter("dmask", [128, 2, NT, 128], BF16, isOutput=False)
    bm_ext = nc.declare_dram_parameter("bmask", [128, 2, NT, NT], FP32, isOutput=False)
    if with_bias:
        b_ext = nc.declare_dram_parameter("bvec", [D], FP32, isOutput=False)
    o_ext = nc.declare_dram_parameter("o", [BPC, 2 * D], FP32, isOutput=True)

    with tile.TileContext(nc) as tc:
        with (
            tc.tile_pool(name="consts", bufs=1) as cpool,
            tc.tile_pool(name="x8p", bufs=4) as x8_pool,
            tc.tile_pool(name="xhp", bufs=4) as xh_pool,
            tc.tile_pool(name="uitt", bufs=3) as uit_pool,
            tc.tile_pool(name="at", bufs=3) as at_pool,
            tc.tile_pool(name="mid", bufs=3) as mid_pool,
            tc.tile_pool(name="sums", bufs=4) as sum_pool,
            tc.tile_pool(name="ps1", bufs=1, space="PSUM") as ps1_pool,
            tc.tile_pool(name="ps2", bufs=1, space="PSUM") as ps2_pool,
        ):
            w8 = cpool.tile([128, 2, 2, D], FP8)
            u8 = cpool.tile([128, 2, 2, D], FP8)
            dmask = cpool.tile([128, 2, NT, 128], BF16)
            bmask = cpool.tile([128, 2, NT, NT], FP32)
            if with_bias:
                bias = cpool.tile([128, NT], FP32)

            def load_consts():
                nc.sync.dma_start(out=w8[:], in_=w8_ext[:])
                nc.sync.dma_start(out=u8[:], in_=u8_ext[:])
                nc.sync.dma_start(out=dmask[:], in_=dm_ext[:])
                nc.sync.dma_start(out=bmask[:], in_=bm_ext[:])
                if with_bias:
                    nc.sync.dma_start(
                        out=bias[:], in_=b_ext.rearrange("(e p) -> p e", p=128)
                    )

            def load(b):
                x8 = x8_pool.tile([128, NT, S], FP8, tag="x8")
                xh = xh_pool.tile([128, NT, S], BF16, tag="xh")
                # spread x loads over 4 gpsimd SWDGE rings (num_swdge_queues=4)
                nc.gpsimd.dma_start(out=x8[:, 0:2, :], in_=x8_ext[b, :, 0:2, :])
                nc.gpsimd.dma_start(out=x8[:, 2:4, :], in_=x8_ext[b, :, 2:4, :])
                nc.gpsimd.dma_start(out=xh[:, 0:2, :], in_=xh_ext[b, :, 0:2, :])
                nc.gpsimd.dma_start(out=xh[:, 2:4, :], in_=xh_ext[b, :, 2:4, :])
                return x8, xh

            def mm1(x8):
                uitt = uit_pool.tile([128, NT, S], FP8, tag="uitt")
                ps1 = ps1_pool.tile([128, NT, S], FP32, tag="ps1")
                for e in range(NT):
                    for j in range(2):
                        nc.tensor.matmul(
                            ps1[:, e, :],
                            lhsT=w8[:, j, :, 128 * e : 128 * (e + 1)],
                            rhs=x8[:, 2 * j : 2 * j + 2, :],
                            start=(j == 0),
                            stop=(j == 1),
                            perf_mode=DR,
                        )
                if with_bias:
                    for e in range(NT):
                        nc.scalar.activation(
                            uitt[:, e, :],
                            ps1[:, e, :],
                            AF.Tanh,
                            bias=bias[:, e : e + 1],
                            scale=float(1.0 / WSCALE),
                        )
                else:
                    nc.scalar.activation(
                        uitt.rearrange("p k s -> p (k s)"),
                        ps1.rearrange("p k s -> p (k s)"),
                        AF.Tanh,
                        scale=float(1.0 / WSCALE),
                    )
                return uitt

            def mm2_exp(uitt):
                aT = at_pool.tile([128, 5, S], BF16, tag="at")
                aflat = aT.rearrange("p k s -> p (k s)")
                ps2 = ps2_pool.tile([128, NT, S], FP32, tag="ps2")
                for f in range(NT):
                    for j in range(2):
                        nc.tensor.matmul(
                            ps2[:, f, :],
                            lhsT=u8[:, j, :, 128 * f : 128 * (f + 1)],
                            rhs=uitt[:, 2 * j : 2 * j + 2, :],
                            start=(j == 0),
                            stop=(j == 1),
                            perf_mode=DR,
                        )
                nc.scalar.activation(
                    aflat[:, 0:2048],
                    ps2.rearrange("p k s -> p (k s)"),
                    AF.Exp,
                    scale=float(1.0 / WSCALE),
                )
                return aT

            def folds(aT):
                """Shared fold tree: bs24[:, 0:16] = block sums,
                bs24[:, 16:24] = (fw, bw) masked diagonal sums."""
                aflat = aT.rearrange("p k s -> p (k s)")
                v = aflat.rearrange("p (g j) -> p g j", j=128)[:, 0:16, :]
                cmb = mid_pool.tile([128, 24, 64], BF16, tag="cmb")
                nc.vector.tensor_tensor(
                    out=cmb[:, 0:16, :], in0=v[:, :, 0:64], in1=v[:, :, 64:128],
                    op=OP.add,
                )
                dv = aflat.rearrange("p (f y) -> p f y", y=640)[:, :, 0:128]
                md = mid_pool.tile([128, 2, NT, 128], BF16, tag="md")
                for d_ in range(2):
                    nc.vector.tensor_tensor(
                        out=md[:, d_], in0=dv, in1=dmask[:, d_], op=OP.mult
                    )
                mdv = md.rearrange("p d f j -> p (d f) j")
                nc.vector.tensor_tensor(
                    out=cmb[:, 16:24, :], in0=mdv[:, :, 0:64], in1=mdv[:, :, 64:128],
                    op=OP.add,
                )
                cm2 = mid_pool.tile([128, 24, 32], BF16, tag="cm2")
                nc.vector.tensor_tensor(
                    out=cm2[:], in0=cmb[:, :, 0:32], in1=cmb[:, :, 32:64], op=OP.add
                )
                bs24 = sum_pool.tile([128, 24], FP32, tag="bs24")
                nc.vector.tensor_reduce(out=bs24[:], in_=cm2[:], axis=AX.X, op=OP.add)
                return bs24

            def xs_folds(xh):
                xs2 = sum_pool.tile([128, 2 * NT], FP32, tag="xs2")
                xa = mid_pool.tile([128, NT, 256], BF16, tag="xa")
                nc.vector.tensor_tensor(
                    out=xa[:], in0=xh[:, :, 0:256], in1=xh[:, :, 256:512], op=OP.add
                )
                xb_ = mid_pool.tile([128, NT, 128], BF16, tag="xb_")
                nc.vector.tensor_tensor(
                    out=xb_[:], in0=xa[:, :, 0:128], in1=xa[:, :, 128:256], op=OP.add
                )
                nc.vector.tensor_reduce(
                    out=xs2[:, 0:NT], in_=xb_[:], axis=AX.X, op=OP.add
                )
                nc.vector.tensor_copy(xs2[:, NT : 2 * NT], xs2[:, 0:NT])
                return xs2

            def asm_out(b, xs2, bs24):
                bsm = mid_pool.tile([128, 2, NT, NT], FP32, tag="bsm")
                for d_ in range(2):
                    nc.vector.tensor_tensor(
                        out=bsm[:, d_],
                        in0=bs24[:, 0:16].rearrange("p (f k) -> p f k", k=NT),
                        in1=bmask[:, d_],
                        op=OP.mult,
                    )
                osb = sum_pool.tile([128, 2 * NT], FP32, tag="osb")
                nc.vector.tensor_reduce(
                    out=osb[:],
                    in_=bsm.rearrange("p d f k -> p (d f) k"),
                    axis=AX.X,
                    op=OP.add,
                )
                o2a = sum_pool.tile([128, 2 * NT], FP32, tag="o2a")
                nc.vector.tensor_tensor(
                    out=o2a[:], in0=osb[:], in1=bs24[:, 16:24], op=OP.add
                )
                o2 = sum_pool.tile([128, 2 * NT], FP32, tag="o2")
                nc.vector.scalar_tensor_tensor(
                    out=o2[:], in0=o2a[:], scalar=EPS, in1=xs2[:],
                    op0=OP.add, op1=OP.mult,
                )
                nc.sync.dma_start(
                    out=o_ext[b].rearrange("(c p) -> p c", p=128), in_=o2[:]
                )

            def body(first_iter):
                state = {}  # b -> (uitt, xh)
                mid = {}  # b -> (xs2, bs24)
                for b in range(BPC + 2):
                    if b >= 2:
                        pb = b - 2
                        asm_out(pb, *mid[pb])
                    if b < BPC:
                        if b == 0 and first_iter:
                            load_consts()
                        x8, xh = load(b)
                        uitt = mm1(x8)
                        state[b] = (uitt, xh)
                    if 1 <= b <= BPC:
                        pb = b - 1
                        uitt, xh = state[pb]
                        aT = mm2_exp(uitt)
                        bs24 = folds(aT)
                        xs2 = xs_folds(xh)
                        mid[pb] = (xs2, bs24)

            if repeat == 1:
                body(True)
            else:
                load_consts()
                with tc.For_i(0, repeat, 1):
                    for _u in range(unroll):
                        body(False)

    nc.finalize()
    return nc


def _e4m3(a):
    import ml_dtypes

    return np.clip(np.asarray(a, np.float32), -240.0, 240.0).astype(
        ml_dtypes.float8_e4m3
    )


def _make_mask_inputs():
    import ml_dtypes

    j = np.arange(128)
    fw = (j[None, :] > j[:, None]).astype(np.float32)
    bw = fw.T
    dmask = np.stack(
        [np.repeat(fw[:, None, :], NT, axis=1), np.repeat(bw[:, None, :], NT, axis=1)]
    ).transpose(1, 0, 2, 3)
    f = np.arange(NT)
    bfw = (f[None, :] > f[:, None]).astype(np.float32)
    bbw = bfw.T
    bmask = np.broadcast_to(np.stack([bfw, bbw])[None], (128, 2, NT, NT))
    return (
        np.ascontiguousarray(dmask.astype(ml_dtypes.bfloat16)),
        np.ascontiguousarray(bmask.astype(np.float32)),
    )


def _make_in_maps(x, W, U, b):
    import ml_dtypes

    x = np.asarray(x, np.float32)
    xt = x.transpose(0, 2, 1).reshape(B, NT, 128, S).transpose(0, 2, 1, 3)
    x8 = np.ascontiguousarray(_e4m3(xt))
    xh = np.ascontiguousarray(xt.astype(ml_dtypes.bfloat16))

    def prep_w(M):
        M8 = _e4m3(np.asarray(M, np.float32) * WSCALE)
        return np.ascontiguousarray(M8.reshape(2, 2, 128, D).transpose(2, 0, 1, 3))

    w8 = prep_w(W)
    u8 = prep_w(U)
    dmask, bmask = _make_mask_inputs()
    base = {"w8": w8, "u8": u8, "dmask": dmask, "bmask": bmask}
    bvec = np.ascontiguousarray(np.asarray(b, np.float32))
    if np.any(bvec):
        base["bvec"] = bvec
    return [
        {"x8": x8[c * BPC : (c + 1) * BPC], "xh": xh[c * BPC : (c + 1) * BPC], **base}
        for c in range(N_CORES)
    ]


def kernel(x, W, U, b):
    from concourse.bass_utils import run_bass_kernel_spmd

    x = np.asarray(x)
    assert x.shape == (B, S, D)
    in_maps = _make_in_maps(x, np.asarray(W), np.asarray(U), np.asarray(b))
    with_bias = "bvec" in in_maps[0]
    key = ("nc", with_bias)
    if key not in _NC_CACHE:
        _NC_CACHE[key] = _build_nc(1, with_bias=with_bias)
    nc = _NC_CACHE[key]

    res = run_bass_kernel_spmd(nc, in_maps, list(range(N_CORES)))
    out = np.concatenate([res.results[c]["o"] for c in range(N_CORES)], axis=0)
    return out.astype(np.float32)


# revision 5
# speedup vs baseline: 1.3028x; 1.1642x over previous
"""Trainium2 Bass kernel for nn_Direction_Attention_layer (sparse_attention), v8.

Math (S == D == 512):
    uit  = tanh(x @ W + b);  a = exp(uit @ U)
    fw_a[d] = EPS + sum_{s>d} a[s,d];  bw_a[d] = EPS + sum_{s<d} a[s,d]
    out = concat(fw_a * xs, bw_a * xs),  xs[d] = sum_s x[s,d]

Sharding: data-parallel over batch B=64 across 8 cores; W/U replicated.

v5 vs v3 (both fp8-DoubleRow GEMMs + merged tanh/exp):
- ALL reduction work on DVE (measured: gpsimd tensor ops are ~4x slower
  than the cost model; independent back-to-back DVE ops run at model rate).
- One shared fold tree for block-sums + masked diagonal ([128,24,64] ->
  [128,24,32] -> [128,24] fp32).
- Stage-decoupled emission so the in-order DVE queue never waits:
  per iteration: asm(b-2) [old deps] -> folds(b-1) [dep exp(b-1)] ->
  xs folds(b-1) [dep xh(b-1), loaded last iteration].
- DMA spread across queues: x8 on sync HWDGE, xh halves on two gpsimd
  SWDGE chains (gpsimd is otherwise idle) - per-queue DMA bandwidth
  (~22GB/s) was a serialization risk at 768KB/iteration.
"""

import sys

sys.path.insert(0, "/opt/trn_rl_repo")

import numpy as np

B, S, D = 64, 512, 512
N_CORES = 8
BPC = B // N_CORES
NT = D // 128  # 4
EPS = 1e-7
WSCALE = 2048.0

_NC_CACHE = {}


def _build_nc(repeat: int = 1, with_bias: bool = False, unroll: int = 1):
    import concourse.bass as bass
    import concourse.tile as tile
    from concourse import bacc, mybir

    FP32 = mybir.dt.float32
    BF16 = mybir.dt.bfloat16
    FP8 = mybir.dt.float8e4
    AX = mybir.AxisListType
    OP = mybir.AluOpType
    AF = mybir.ActivationFunctionType
    DR = mybir.MatmulPerfMode.DoubleRow

    nc = bacc.Bacc("TRN2", target_bir_lowering=False, debug=False, num_devices=N_CORES, num_swdge_queues=4)

    x8_ext = nc.declare_dram_parameter("x8", [BPC, 128, NT, S], FP8, isOutput=False)
    xh_ext = nc.declare_dram_parameter("xh", [BPC, 128, NT, S], BF16, isOutput=False)
    w8_ext = nc.declare_dram_parameter("w8", [128, 2, 2, D], FP8, isOutput=False)
    u8_ext = nc.declare_dram_parameter("u8", [128, 2, 2, D], FP8, isOutput=False)
    dm_ext = nc.declare_dram_parameter("dmask", [128, 2, NT, 128], BF16, isOutput=False)
    bm_ext = nc.declare_dram_parameter("bmask", [128, 2, NT, NT], FP32, isOutput=False)
    if with_bias:
        b_ext = nc.declare_dram_parameter("bvec", [D], FP32, isOutput=False)
    o_ext = nc.declare_dram_parameter("o", [BPC, 2 * D], FP32, isOutput=True)

    with tile.TileContext(nc) as tc:
        with (
            tc.tile_pool(name="consts", bufs=1) as cpool,
            tc.tile_pool(name="x8p", bufs=4) as x8_pool,
            tc.tile_pool(name="xhp", bufs=4) as xh_pool,
            tc.tile_pool(name="uitt", bufs=3) as uit_pool,
            tc.tile_pool(name="at", bufs=3) as at_pool,
            tc.tile_pool(name="mid", bufs=3) as mid_pool,
            tc.tile_pool(name="sums", bufs=4) as sum_pool,
            tc.tile_pool(name="ps1", bufs=1, space="PSUM") as ps1_pool,
            tc.tile_pool(name="ps2", bufs=1, space="PSUM") as ps2_pool,
        ):
            w8 = cpool.tile([128, 2, 2, D], FP8)
            u8 = cpool.tile([128, 2, 2, D], FP8)
            dmask = cpool.tile([128, 2, NT, 128], BF16)
            bmask = cpool.tile([128, 2, NT, NT], FP32)
            if with_bias:
                bias = cpool.tile([128, NT], FP32)

            def load_consts():
                nc.sync.dma_start(out=w8[:], in_=w8_ext[:])
                nc.sync.dma_start(out=u8[:], in_=u8_ext[:])
                nc.sync.dma_start(out=dmask[:], in_=dm_ext[:])
                nc.sync.dma_start(out=bmask[:], in_=bm_ext[:])
                if with_bias:
                    nc.sync.dma_start(
                        out=bias[:], in_=b_ext.rearrange("(e p) -> p e", p=128)
                    )

            def load(b):
                x8 = x8_pool.tile([128, NT, S], FP8, tag="x8")
                xh = xh_pool.tile([128, NT, S], BF16, tag="xh")
                # spread x loads over 4 gpsimd SWDGE rings (num_swdge_queues=4)
                nc.gpsimd.dma_start(out=x8[:, 0:2, :], in_=x8_ext[b, :, 0:2, :])
                nc.gpsimd.dma_start(out=x8[:, 2:4, :], in_=x8_ext[b, :, 2:4, :])
                nc.gpsimd.dma_start(out=xh[:, 0:2, :], in_=xh_ext[b, :, 0:2, :])
                nc.gpsimd.dma_start(out=xh[:, 2:4, :], in_=xh_ext[b, :, 2:4, :])
                return x8, xh

            def mm1(x8):
                uitt = uit_pool.tile([128, NT, S], FP8, tag="uitt")
                ps1 = ps1_pool.tile([128, NT, S], FP32, tag="ps1")
                for e in range(NT):
                    for j in range(2):
                        nc.tensor.matmul(
                            ps1[:, e, :],
                            lhsT=w8[:, j, :, 128 * e : 128 * (e + 1)],
                            rhs=x8[:, 2 * j : 2 * j + 2, :],
                            start=(j == 0),
                            stop=(j == 1),
                            perf_mode=DR,
                        )
                if with_bias:
                    for e in range(NT):
                        nc.scalar.activation(
                            uitt[:, e, :],
                            ps1[:, e, :],
                            AF.Tanh,
                            bias=bias[:, e : e + 1],
                            scale=float(1.0 / WSCALE),
                        )
                else:
                    nc.scalar.activation(
                        uitt.rearrange("p k s -> p (k s)"),
                        ps1.rearrange("p k s -> p (k s)"),
                        AF.Tanh,
                        scale=float(1.0 / WSCALE),
                    )
                return uitt

            def mm2_exp(uitt):
                aT = at_pool.tile([128, 5, S], BF16, tag="at")
                aflat = aT.rearrange("p k s -> p (k s)")
                ps2 = ps2_pool.tile([128, NT, S], FP32, tag="ps2")
                for f in range(NT):
                    for j in range(2):
                        nc.tensor.matmul(
                            ps2[:, f, :],
                            lhsT=u8[:, j, :, 128 * f : 128 * (f + 1)],
                            rhs=uitt[:, 2 * j : 2 * j + 2, :],
                            start=(j == 0),
                            stop=(j == 1),
                            perf_mode=DR,
                        )
                nc.scalar.activation(
                    aflat[:, 0:2048],
                    ps2.rearrange("p k s -> p (k s)"),
                    AF.Exp,
                    scale=float(1.0 / WSCALE),
                )
                return aT

            def folds(aT):
                """Shared fold tree: bs24[:, 0:16] = block sums,
                bs24[:, 16:24] = (fw, bw) masked diagonal sums."""
                aflat = aT.rearrange("p k s -> p (k s)")
                v = aflat.rearrange("p (g j) -> p g j", j=128)[:, 0:16, :]
                cmb = mid_pool.tile([128, 24, 64], BF16, tag="cmb")
                nc.vector.tensor_tensor(
                    out=cmb[:, 0:16, :], in0=v[:, :, 0:64], in1=v[:, :, 64:128],
                    op=OP.add,
                )
                dv = aflat.rearrange("p (f y) -> p f y", y=640)[:, :, 0:128]
                md = mid_pool.tile([128, 2, NT, 128], BF16, tag="md")
                for d_ in range(2):
                    nc.vector.tensor_tensor(
                        out=md[:, d_], in0=dv, in1=dmask[:, d_], op=OP.mult
                    )
                mdv = md.rearrange("p d f j -> p (d f) j")
                nc.vector.tensor_tensor(
                    out=cmb[:, 16:24, :], in0=mdv[:, :, 0:64], in1=mdv[:, :, 64:128],
                    op=OP.add,
                )
                cm2 = mid_pool.tile([128, 24, 32], BF16, tag="cm2")
                nc.vector.tensor_tensor(
                    out=cm2[:], in0=cmb[:, :, 0:32], in1=cmb[:, :, 32:64], op=OP.add
                )
                bs24 = sum_pool.tile([128, 24], FP32, tag="bs24")
                nc.vector.tensor_reduce(out=bs24[:], in_=cm2[:], axis=AX.X, op=OP.add)
                return bs24

            def xs_folds(xh):
                xs2 = sum_pool.tile([128, 2 * NT], FP32, tag="xs2")
                xa = mid_pool.tile([128, NT, 256], BF16, tag="xa")
                nc.vector.tensor_tensor(
                    out=xa[:], in0=xh[:, :, 0:256], in1=xh[:, :, 256:512], op=OP.add
                )
                xb_ = mid_pool.tile([128, NT, 128], BF16, tag="xb_")
                nc.vector.tensor_tensor(
                    out=xb_[:], in0=xa[:, :, 0:128], in1=xa[:, :, 128:256], op=OP.add
                )
                nc.vector.tensor_reduce(
                    out=xs2[:, 0:NT], in_=xb_[:], axis=AX.X, op=OP.add
                )
                nc.vector.tensor_copy(xs2[:, NT : 2 * NT], xs2[:, 0:NT])
                return xs2

            def asm_out(b, xs2, bs24):
                bsm = mid_pool.tile([128, 2, NT, NT], FP32, tag="bsm")
                for d_ in range(2):
                    nc.vector.tensor_tensor(
                        out=bsm[:, d_],
                        in0=bs24[:, 0:16].rearrange("p (f k) -> p f k", k=NT),
                        in1=bmask[:, d_],
                        op=OP.mult,
                    )
                osb = sum_pool.tile([128, 2 * NT], FP32, tag="osb")
                nc.vector.tensor_reduce(
                    out=osb[:],
                    in_=bsm.rearrange("p d f k -> p (d f) k"),
                    axis=AX.X,
                    op=OP.add,
                )
                o2a = sum_pool.tile([128, 2 * NT], FP32, tag="o2a")
                nc.vector.tensor_tensor(
                    out=o2a[:], in0=osb[:], in1=bs24[:, 16:24], op=OP.add
                )
                o2 = sum_pool.tile([128, 2 * NT], FP32, tag="o2")
                nc.vector.scalar_tensor_tensor(
                    out=o2[:], in0=o2a[:], scalar=EPS, in1=xs2[:],
                    op0=OP.add, op1=OP.mult,
                )
                nc.sync.dma_start(
                    out=o_ext[b].rearrange("(c p) -> p c", p=128), in_=o2[:]
                )

            def body(first_iter):
                state = {}  # b -> (uitt, xh)
                mid = {}  # b -> (xs2, bs24)
                for b in range(BPC + 2):
                    if b >= 2:
                        pb = b - 2
                        asm_out(pb, *mid[pb])
                    if b < BPC:
                        if b == 0 and first_iter:
                            load_consts()
                        x8, xh = load(b)
                        uitt = mm1(x8)
                        state[b] = (uitt, xh)
                    if 1 <= b <= BPC:
                        pb = b - 1
                        uitt, xh = state[pb]
                        xs2 = xs_folds(xh)
                        aT = mm2_exp(uitt)
                        bs24 = folds(aT)
                        mid[pb] = (xs2, bs24)

            if repeat == 1:
                body(True)
            else:
                load_consts()
                with tc.For_i(0, repeat, 1):
                    for _u in range(unroll):
                        body(False)

    nc.finalize()
    return nc


def _e4m3(a):
    import ml_dtypes

    return np.clip(np.asarray(a, np.float32), -240.0, 240.0).astype(
        ml_dtypes.float8_e4m3
    )


def _make_mask_inputs():
    import ml_dtypes

    j = np.arange(128)
    fw = (j[None, :] > j[:, None]).astype(np.float32)
    bw = fw.T
    dmask = np.stack(
        [np.repeat(fw[:, None, :], NT, axis=1), np.repeat(bw[:, None, :], NT, axis=1)]
    ).transpose(1, 0, 2, 3)
    f = np.arange(NT)
    bfw = (f[None, :] > f[:, None]).astype(np.float32)
    bbw = bfw.T
    bmask = np.broadcast_to(np.stack([bfw, bbw])[None], (128, 2, NT, NT))
    return (
        np.ascontiguousarray(dmask.astype(ml_dtypes.bfloat16)),
        np.ascontiguousarray(bmask.astype(np.float32)),
    )


def _make_in_maps(x, W, U, b):
    import ml_dtypes

    x = np.asarray(x, np.float32)
    xt = x.transpose(0, 2, 1).reshape(B, NT, 128, S).transpose(0, 2, 1, 3)
    x8 = np.ascontiguousarray(_e4m3(xt))
    xh = np.ascontiguousarray(xt.astype(ml_dtypes.bfloat16))

    def prep_w(M):
        M8 = _e4m3(np.asarray(M, np.float32) * WSCALE)
        return np.ascontiguousarray(M8.reshape(2, 2, 128, D).transpose(2, 0, 1, 3))

    w8 = prep_w(W)
    u8 = prep_w(U)
    dmask, bmask = _make_mask_inputs()
    base = {"w8": w8, "u8": u8, "dmask": dmask, "bmask": bmask}
    bvec = np.ascontiguousarray(np.asarray(b, np.float32))
    if np.any(bvec):
        base["bvec"] = bvec
    return [
        {"x8": x8[c * BPC : (c + 1) * BPC], "xh": xh[c * BPC : (c + 1) * BPC], **base}
        for c in range(N_CORES)
    ]


def kernel(x, W, U, b):
    from concourse.bass_utils import run_bass_kernel_spmd

    x = np.asarray(x)
    assert x.shape == (B, S, D)
    in_maps = _make_in_maps(x, np.asarray(W), np.asarray(U), np.asarray(b))
    with_bias = "bvec" in in_maps[0]
    key = ("nc", with_bias)
    if key not in _NC_CACHE:
        _NC_CACHE[key] = _build_nc(1, with_bias=with_bias)
    nc = _NC_CACHE[key]

    res = run_bass_kernel_spmd(nc, in_maps, list(range(N_CORES)))
    out = np.concatenate([res.results[c]["o"] for c in range(N_CORES)], axis=0)
    return out.astype(np.float32)


# revision 6
# speedup vs baseline: 1.3792x; 1.0586x over previous
"""Trainium2 Bass kernel for nn_Direction_Attention_layer (sparse_attention), v9.

Math (S == D == 512):
    uit  = tanh(x @ W + b);  a = exp(uit @ U)
    fw_a[d] = EPS + sum_{s>d} a[s,d];  bw_a[d] = EPS + sum_{s<d} a[s,d]
    out = concat(fw_a * xs, bw_a * xs),  xs[d] = sum_s x[s,d]

Sharding: data-parallel over batch B=64 across 8 cores; W/U replicated.

v5 vs v3 (both fp8-DoubleRow GEMMs + merged tanh/exp):
- ALL reduction work on DVE (measured: gpsimd tensor ops are ~4x slower
  than the cost model; independent back-to-back DVE ops run at model rate).
- One shared fold tree for block-sums + masked diagonal ([128,24,64] ->
  [128,24,32] -> [128,24] fp32).
- Stage-decoupled emission so the in-order DVE queue never waits:
  per iteration: asm(b-2) [old deps] -> folds(b-1) [dep exp(b-1)] ->
  xs folds(b-1) [dep xh(b-1), loaded last iteration].
- DMA spread across queues: x8 on sync HWDGE, xh halves on two gpsimd
  SWDGE chains (gpsimd is otherwise idle) - per-queue DMA bandwidth
  (~22GB/s) was a serialization risk at 768KB/iteration.
"""

import sys

sys.path.insert(0, "/opt/trn_rl_repo")

import numpy as np

B, S, D = 64, 512, 512
N_CORES = 8
BPC = B // N_CORES
NT = D // 128  # 4
EPS = 1e-7
WSCALE = 2048.0

_NC_CACHE = {}


def _build_nc(repeat: int = 1, with_bias: bool = False, unroll: int = 1):
    import concourse.bass as bass
    import concourse.tile as tile
    from concourse import bacc, mybir

    FP32 = mybir.dt.float32
    BF16 = mybir.dt.bfloat16
    FP8 = mybir.dt.float8e4
    AX = mybir.AxisListType
    OP = mybir.AluOpType
    AF = mybir.ActivationFunctionType
    DR = mybir.MatmulPerfMode.DoubleRow

    nc = bacc.Bacc("TRN2", target_bir_lowering=False, debug=False, num_devices=N_CORES, num_swdge_queues=4)

    x8_ext = nc.declare_dram_parameter("x8", [BPC, 128, NT, S], FP8, isOutput=False)
    xh_ext = nc.declare_dram_parameter("xh", [BPC, 128, NT, S], BF16, isOutput=False)
    w8_ext = nc.declare_dram_parameter("w8", [128, 2, 2, D], FP8, isOutput=False)
    u8_ext = nc.declare_dram_parameter("u8", [128, 2, 2, D], FP8, isOutput=False)
    dm_ext = nc.declare_dram_parameter("dmask", [128, 2, NT, 128], BF16, isOutput=False)
    bm_ext = nc.declare_dram_parameter("bmask", [128, 2, NT, NT], FP32, isOutput=False)
    if with_bias:
        b_ext = nc.declare_dram_parameter("bvec", [D], FP32, isOutput=False)
    o_ext = nc.declare_dram_parameter("o", [BPC, 2 * D], FP32, isOutput=True)

    with tile.TileContext(nc) as tc:
        with (
            tc.tile_pool(name="consts", bufs=1) as cpool,
            tc.tile_pool(name="x8p", bufs=4) as x8_pool,
            tc.tile_pool(name="xhp", bufs=4) as xh_pool,
            tc.tile_pool(name="uitt", bufs=3) as uit_pool,
            tc.tile_pool(name="at", bufs=3) as at_pool,
            tc.tile_pool(name="mid", bufs=3) as mid_pool,
            tc.tile_pool(name="sums", bufs=4) as sum_pool,
            tc.tile_pool(name="ps1", bufs=1, space="PSUM") as ps1_pool,
            tc.tile_pool(name="ps2", bufs=1, space="PSUM") as ps2_pool,
        ):
            w8 = cpool.tile([128, 2, 2, D], FP8)
            u8 = cpool.tile([128, 2, 2, D], FP8)
            dmask = cpool.tile([128, 2, NT, 128], BF16)
            bmask = cpool.tile([128, 2, NT, NT], FP32)
            if with_bias:
                bias = cpool.tile([128, NT], FP32)

            def load_consts():
                nc.sync.dma_start(out=w8[:], in_=w8_ext[:])
                nc.sync.dma_start(out=u8[:], in_=u8_ext[:])
                nc.sync.dma_start(out=dmask[:], in_=dm_ext[:])
                nc.sync.dma_start(out=bmask[:], in_=bm_ext[:])
                if with_bias:
                    nc.sync.dma_start(
                        out=bias[:], in_=b_ext.rearrange("(e p) -> p e", p=128)
                    )

            def load(b):
                x8 = x8_pool.tile([128, NT, S], FP8, tag="x8")
                xh = xh_pool.tile([128, NT, S], BF16, tag="xh")
                # three independent DMA paths: sync HWDGE, ACT HWDGE,
                # gpsimd SWDGE ring (plain dma_start always uses ring 0)
                nc.sync.dma_start(out=x8[:], in_=x8_ext[b])
                nc.scalar.dma_start(out=xh[:, 0:2, :], in_=xh_ext[b, :, 0:2, :])
                nc.gpsimd.dma_start(out=xh[:, 2:4, :], in_=xh_ext[b, :, 2:4, :])
                return x8, xh

            def mm1(x8):
                uitt = uit_pool.tile([128, NT, S], FP8, tag="uitt")
                ps1 = ps1_pool.tile([128, NT, S], FP32, tag="ps1")
                for e in range(NT):
                    for j in range(2):
                        nc.tensor.matmul(
                            ps1[:, e, :],
                            lhsT=w8[:, j, :, 128 * e : 128 * (e + 1)],
                            rhs=x8[:, 2 * j : 2 * j + 2, :],
                            start=(j == 0),
                            stop=(j == 1),
                            perf_mode=DR,
                        )
                if with_bias:
                    for e in range(NT):
                        nc.scalar.activation(
                            uitt[:, e, :],
                            ps1[:, e, :],
                            AF.Tanh,
                            bias=bias[:, e : e + 1],
                            scale=float(1.0 / WSCALE),
                        )
                else:
                    nc.scalar.activation(
                        uitt.rearrange("p k s -> p (k s)"),
                        ps1.rearrange("p k s -> p (k s)"),
                        AF.Tanh,
                        scale=float(1.0 / WSCALE),
                    )
                return uitt

            def mm2_exp(uitt):
                aT = at_pool.tile([128, 5, S], BF16, tag="at")
                aflat = aT.rearrange("p k s -> p (k s)")
                ps2 = ps2_pool.tile([128, NT, S], FP32, tag="ps2")
                for f in range(NT):
                    for j in range(2):
                        nc.tensor.matmul(
                            ps2[:, f, :],
                            lhsT=u8[:, j, :, 128 * f : 128 * (f + 1)],
                            rhs=uitt[:, 2 * j : 2 * j + 2, :],
                            start=(j == 0),
                            stop=(j == 1),
                            perf_mode=DR,
                        )
                nc.scalar.activation(
                    aflat[:, 0:2048],
                    ps2.rearrange("p k s -> p (k s)"),
                    AF.Exp,
                    scale=float(1.0 / WSCALE),
                )
                return aT

            def folds(aT):
                """Shared fold tree: bs24[:, 0:16] = block sums,
                bs24[:, 16:24] = (fw, bw) masked diagonal sums."""
                aflat = aT.rearrange("p k s -> p (k s)")
                v = aflat.rearrange("p (g j) -> p g j", j=128)[:, 0:16, :]
                cmb = mid_pool.tile([128, 24, 64], BF16, tag="cmb")
                nc.vector.tensor_tensor(
                    out=cmb[:, 0:16, :], in0=v[:, :, 0:64], in1=v[:, :, 64:128],
                    op=OP.add,
                )
                dv = aflat.rearrange("p (f y) -> p f y", y=640)[:, :, 0:128]
                md = mid_pool.tile([128, 2, NT, 128], BF16, tag="md")
                for d_ in range(2):
                    nc.vector.tensor_tensor(
                        out=md[:, d_], in0=dv, in1=dmask[:, d_], op=OP.mult
                    )
                mdv = md.rearrange("p d f j -> p (d f) j")
                nc.vector.tensor_tensor(
                    out=cmb[:, 16:24, :], in0=mdv[:, :, 0:64], in1=mdv[:, :, 64:128],
                    op=OP.add,
                )
                cm2 = mid_pool.tile([128, 24, 32], BF16, tag="cm2")
                nc.vector.tensor_tensor(
                    out=cm2[:], in0=cmb[:, :, 0:32], in1=cmb[:, :, 32:64], op=OP.add
                )
                bs24 = sum_pool.tile([128, 24], FP32, tag="bs24")
                nc.vector.tensor_reduce(out=bs24[:], in_=cm2[:], axis=AX.X, op=OP.add)
                return bs24

            def xs_folds(xh):
                xs2 = sum_pool.tile([128, 2 * NT], FP32, tag="xs2")
                xa = mid_pool.tile([128, NT, 256], BF16, tag="xa")
                nc.vector.tensor_tensor(
                    out=xa[:], in0=xh[:, :, 0:256], in1=xh[:, :, 256:512], op=OP.add
                )
                xb_ = mid_pool.tile([128, NT, 128], BF16, tag="xb_")
                nc.vector.tensor_tensor(
                    out=xb_[:], in0=xa[:, :, 0:128], in1=xa[:, :, 128:256], op=OP.add
                )
                nc.vector.tensor_reduce(
                    out=xs2[:, 0:NT], in_=xb_[:], axis=AX.X, op=OP.add
                )
                nc.vector.tensor_copy(xs2[:, NT : 2 * NT], xs2[:, 0:NT])
                return xs2

            def asm_out(b, xs2, bs24):
                bsm = mid_pool.tile([128, 2, NT, NT], FP32, tag="bsm")
                for d_ in range(2):
                    nc.vector.tensor_tensor(
                        out=bsm[:, d_],
                        in0=bs24[:, 0:16].rearrange("p (f k) -> p f k", k=NT),
                        in1=bmask[:, d_],
                        op=OP.mult,
                    )
                osb = sum_pool.tile([128, 2 * NT], FP32, tag="osb")
                nc.vector.tensor_reduce(
                    out=osb[:],
                    in_=bsm.rearrange("p d f k -> p (d f) k"),
                    axis=AX.X,
                    op=OP.add,
                )
                o2a = sum_pool.tile([128, 2 * NT], FP32, tag="o2a")
                nc.vector.tensor_tensor(
                    out=o2a[:], in0=osb[:], in1=bs24[:, 16:24], op=OP.add
                )
                o2 = sum_pool.tile([128, 2 * NT], FP32, tag="o2")
                nc.vector.scalar_tensor_tensor(
                    out=o2[:], in0=o2a[:], scalar=EPS, in1=xs2[:],
                    op0=OP.add, op1=OP.mult,
                )
                nc.sync.dma_start(
                    out=o_ext[b].rearrange("(c p) -> p c", p=128), in_=o2[:]
                )

            def body(first_iter):
                state = {}  # b -> (uitt, xh)
                mid = {}  # b -> (xs2, bs24)
                for b in range(BPC + 2):
                    if b >= 2:
                        pb = b - 2
                        asm_out(pb, *mid[pb])
                    if b < BPC:
                        if b == 0 and first_iter:
                            load_consts()
                        x8, xh = load(b)
                        uitt = mm1(x8)
                        state[b] = (uitt, xh)
                    if 1 <= b <= BPC:
                        pb = b - 1
                        uitt, xh = state[pb]
                        xs2 = xs_folds(xh)
                        aT = mm2_exp(uitt)
                        bs24 = folds(aT)
                        mid[pb] = (xs2, bs24)

            if repeat == 1:
                body(True)
            else:
                load_consts()
                with tc.For_i(0, repeat, 1):
                    for _u in range(unroll):
                        body(False)

    nc.finalize()
    return nc


def _e4m3(a):
    import ml_dtypes

    return np.clip(np.asarray(a, np.float32), -240.0, 240.0).astype(
        ml_dtypes.float8_e4m3
    )


def _make_mask_inputs():
    import ml_dtypes

    j = np.arange(128)
    fw = (j[None, :] > j[:, None]).astype(np.float32)
    bw = fw.T
    dmask = np.stack(
        [np.repeat(fw[:, None, :], NT, axis=1), np.repeat(bw[:, None, :], NT, axis=1)]
    ).transpose(1, 0, 2, 3)
    f = np.arange(NT)
    bfw = (f[None, :] > f[:, None]).astype(np.float32)
    bbw = bfw.T
    bmask = np.broadcast_to(np.stack([bfw, bbw])[None], (128, 2, NT, NT))
    return (
        np.ascontiguousarray(dmask.astype(ml_dtypes.bfloat16)),
        np.ascontiguousarray(bmask.astype(np.float32)),
    )


def _make_in_maps(x, W, U, b):
    import ml_dtypes

    x = np.asarray(x, np.float32)
    xt = x.transpose(0, 2, 1).reshape(B, NT, 128, S).transpose(0, 2, 1, 3)
    x8 = np.ascontiguousarray(_e4m3(xt))
    xh = np.ascontiguousarray(xt.astype(ml_dtypes.bfloat16))

    def prep_w(M):
        M8 = _e4m3(np.asarray(M, np.float32) * WSCALE)
        return np.ascontiguousarray(M8.reshape(2, 2, 128, D).transpose(2, 0, 1, 3))

    w8 = prep_w(W)
    u8 = prep_w(U)
    dmask, bmask = _make_mask_inputs()
    base = {"w8": w8, "u8": u8, "dmask": dmask, "bmask": bmask}
    bvec = np.ascontiguousarray(np.asarray(b, np.float32))
    if np.any(bvec):
        base["bvec"] = bvec
    return [
        {"x8": x8[c * BPC : (c + 1) * BPC], "xh": xh[c * BPC : (c + 1) * BPC], **base}
        for c in range(N_CORES)
    ]


def kernel(x, W, U, b):
    from concourse.bass_utils import run_bass_kernel_spmd

    x = np.asarray(x)
    assert x.shape == (B, S, D)
    in_maps = _make_in_maps(x, np.asarray(W), np.asarray(U), np.asarray(b))
    with_bias = "bvec" in in_maps[0]
    key = ("nc", with_bias)
    if key not in _NC_CACHE:
        _NC_CACHE[key] = _build_nc(1, with_bias=with_bias)
    nc = _NC_CACHE[key]

    res = run_bass_kernel_spmd(nc, in_maps, list(range(N_CORES)))
    out = np.concatenate([res.results[c]["o"] for c in range(N_CORES)], axis=0)
    return out.astype(np.float32)
